# revision 19
# baseline (speedup 1.0000x reference)
"""NeuralMMU Trainium2 kernel (ship-logits design).

Per core (131072 addrs = 256 blocks of 512; device computes blocks 0-251,
the host computes the final 4 blocks exactly in f64 alongside the fixup):
  1. Host unpacks addresses into fp16 bit planes [128, cols]: partition
     32*(b%4)+k holds bit k of block b; one [128, 2048] DMA per 16 blocks.
     The first DMA also carries the fp16 weights (W1 replicated per band,
     W2, b1) so the whole head is a single short DMA chain.
  2. L1 per block: one fp16 matmul (K=32, 1 cyc/row): W1 @ bits ->
     PSUM hpre slot (f32 accumulate).  hpre slots cover 3 blocks
     ([128, 1536] = 3 PSUM banks, double buffered).
  3. ACT: exact Gelu(+b1) per slot, [128, 1536] PSUM -> SBUF fp16 h.
     This is the bottleneck engine (~124.5us busy of the ~132.6us total;
     the activation cost model is (N + 222 init)/1.2GHz per instruction,
     and PSUM capacity (8 banks) caps the slot size at 3 banks once hpre
     is double-buffered and l2o ping-pongs in the last 2 banks).
  4. L2 per block: one fp16 matmul h @ W2ext -> l2o [32 band, 512]
     (col tile_position), 4 blocks share a PSUM bank pair (2 banks,
     ping-pong).
  5. DVE copies l2o [128, 512] f32 PSUM -> SBUF; one [128, 1024] f32
     DMA per 2 groups ships raw logits to DRAM (the final group ships
     alone so the kernel tail is short).
  6. Host adds b2, thresholds logits at 0.5 and packs the 26 bits;
     addrs with any logit within 4e-3 of 0.5 - plus the last 2048 addrs
     per core - are recomputed exactly in f64 (vectorized numpy).  The
     fp16 rounding error is ~2e-4 rms in logit space, so the band covers
     it with 20x margin and the output matches the f32 reference up to
     ~1e-7 ties.

PSUM budget: 2*3 (hpre) + 2*1 (l2o) = 8 banks.
Software pipeline per gelu-slot s: gelu(s) | L2+copy of slot s-1 |
L1 of slot s+2 (hpre double-buffering throttles L1 to one slot ahead in
execution), so each gelu's input is ready a full slot early and the ACT
engine runs back-to-back.  Dummy matmuls at t=0 warm the PE p-state; a
dummy activation pulls the gelu table load off the critical path.
"""

import math

import numpy as np
from contextlib import ExitStack

import concourse.bass as bass
import concourse.mybir as mybir
import concourse.tile as tile
from concourse import bacc, bass_utils

B = 1_048_576
NCORES = 8
PER = B // NCORES          # 131072 addrs per core
BLK = 512                  # addrs per block (one matmul)
NB = PER // BLK            # 256 blocks per core
NB_DEV = 252               # blocks computed on device (last 4 on host)
GIN_B = 16                 # blocks per input DMA ([128, 2048] fp16)
NGRP_IN = NB // GIN_B      # 16 input DMAs
NGROUP = NB_DEV // 4       # 63 logit groups (one l2o bank each)
GOUT_G = 2                 # groups per output DMA ([128, 1024] f32)
NOUT = (NGROUP + GOUT_G - 1) // GOUT_G   # 32 output DMA slots
CW = 162                   # fp16 cols of weights in the head DMA
# gelu slot sizes in blocks (sum 252): tiny head slot so the ACT engine
# starts early, small tail slot so the kernel drains fast.
SLOT_SIZES = [1] + [3] * 83 + [2]
NS = len(SLOT_SIZES)       # 85

_SLOT_OF = []
for _s, _sz in enumerate(SLOT_SIZES):
    for _p in range(_sz):
        _SLOT_OF.append((_s, _p))
_SLOT_BLOCKS = [[] for _ in range(NS)]
for _b, (_s, _p) in enumerate(_SLOT_OF):
    _SLOT_BLOCKS[_s].append(_b)

F32 = mybir.dt.float32
F16 = mybir.dt.float16
AF = mybir.ActivationFunctionType

FIX_BAND = 4e-3            # host recomputes addrs with |logit-0.5| < FIX_BAND


def build_nc() -> bass.Bass:
    nc = bacc.Bacc("TRN2")

    bp = nc.dram_tensor("bp", [NGRP_IN, 128, (GIN_B // 4) * BLK], F16,
                        kind="ExternalInput")
    # Head DMA payload: blocks 0-3 (cols 0:512) + fp16 weights (cols 512:512+CW)
    bp0 = nc.dram_tensor("bp0", [128, BLK + CW], F16, kind="ExternalInput")
    outp = nc.dram_tensor("outp", [NOUT, 128, GOUT_G * BLK], F32,
                          kind="ExternalOutput")

    with ExitStack() as ctx:
        tc = ctx.enter_context(tile.TileContext(nc))
        const = ctx.enter_context(tc.tile_pool(name="const", bufs=1))
        rp = ctx.enter_context(tc.tile_pool(name="rp", bufs=2))
        hp = ctx.enter_context(tc.tile_pool(name="hp", bufs=3))
        lop = ctx.enter_context(tc.tile_pool(name="lop", bufs=2))
        hprep = ctx.enter_context(tc.tile_pool(name="hprep", bufs=2, space="PSUM"))
        l2p = ctx.enter_context(tc.tile_pool(name="l2p", bufs=2, space="PSUM"))

        R = {}                     # input-group index -> tile
        hpre_t = {}                # slot -> PSUM tile
        h_t = {}                   # slot -> SBUF tile
        l2o_t = {}                 # group -> PSUM tile
        lout_t = {}                # out-DMA index -> SBUF tile

        R0x = const.tile([128, BLK + CW], F16, name="R0x")
        nc.sync.dma_start(R0x[:], bp0[:])
        w1r = R0x[:, BLK:BLK + 128]       # W1 fp16, replicated x4 bands
        w2s = R0x[:, BLK + 128:BLK + 160]  # W2[:, :26] fp16, padded to 32
        b1c = R0x[:, BLK + 160:BLK + 162].bitcast(F32)  # b1 f32 [128, 1]

        R0b = rp.tile([128, 3 * BLK], F16, name="R0b", tag="r0b")
        nc.sync.dma_start(R0b[:], bp[0, :, BLK:4 * BLK])

        # PE p-state warm-up: dummy matmuls on a zeroed tile keep the PE
        # busy from ~t=0 until the first real L1 (which lands right after
        # the single head DMA), so real matmuls start past the cold clock.
        wz = const.tile([32, BLK], F32, name="wz")
        nc.vector.memset(wz[:], 0.0)
        # Dummy first activation with no DMA deps: bacc inserts the gelu
        # table load before it, so the 1.3us load runs at ~1us instead of
        # waiting behind gelu(0)'s head-DMA semaphore.
        wgelu = const.tile([32, 1], F32, name="wgelu")
        nc.scalar.activation(wgelu[:], wz[:, 0:1], AF.Gelu, bias=0.0, scale=1.0)
        hpre_t[0] = hprep.tile([128, 3 * BLK], F32, name="hpre")
        for i in range(4):
            nc.tensor.matmul(
                hpre_t[0][:, BLK:2 * BLK][0:128, 0:128],
                wz[:, 0:128], wz[:, 0:128],
                start=True, stop=True, tile_position=(0, 0),
            )

        def dma_in(k):
            t = rp.tile([128, (GIN_B // 4) * BLK], F16)
            nc.sync.dma_start(t[:], bp[k])
            R[k] = t

        def l1(b):
            k, l = divmod(b, GIN_B)
            if l == 0 and k + 1 < NGRP_IN:
                dma_in(k + 1)
            band = b % 4
            s, pos = _SLOT_OF[b]
            if pos == 0 and s not in hpre_t:
                hpre_t[s] = hprep.tile([128, 3 * BLK], F32, name="hpre")
            col = (l // 4) * BLK
            if k == 0 and l < 4:
                src = R0x[32 * band:32 * band + 32, 0:BLK]
            elif k == 0:
                src = R0b[32 * band:32 * band + 32, col - BLK:col]
            else:
                src = R[k][32 * band:32 * band + 32, col:col + BLK]
            nc.tensor.matmul(
                hpre_t[s][:, pos * BLK:(pos + 1) * BLK],
                w1r[32 * band:32 * band + 32, :],
                src,
                start=True, stop=True, tile_position=(32 * band, 0),
            )

        def gelu(s):
            n = SLOT_SIZES[s] * BLK
            ht = hp.tile([128, 3 * BLK], F16)
            nc.scalar.activation(ht[:, 0:n], hpre_t[s][:, 0:n], AF.Gelu,
                                 bias=b1c, scale=1.0)
            h_t[s] = ht

        def l2(b):
            band = b % 4
            g = b // 4
            s, pos = _SLOT_OF[b]
            if band == 0:
                l2o_t[g] = l2p.tile([128, BLK], F32, name="l2o")
            nc.tensor.matmul(
                l2o_t[g][32 * band:32 * band + 32, :],
                w2s,
                h_t[s][:, pos * BLK:(pos + 1) * BLK],
                start=True, stop=True, tile_position=(0, 32 * band),
            )
            if band == 3:
                o, j = divmod(g, GOUT_G)
                if j == 0:
                    lout_t[o] = lop.tile([128, GOUT_G * BLK], F32, name="lout")
                nc.vector.tensor_copy(
                    lout_t[o][:, j * BLK:(j + 1) * BLK], l2o_t[g][:]
                )
                if g == NGROUP - 1:
                    # The final group ships alone so the kernel tail only
                    # waits on a short [128, 512] DMA.
                    nc.sync.dma_start(
                        outp[o][:, j * BLK:(j + 1) * BLK],
                        lout_t[o][:, j * BLK:(j + 1) * BLK],
                    )
                elif j == GOUT_G - 1:
                    nc.sync.dma_start(outp[o], lout_t[o][:])

        # L1 runs two slots ahead of gelu (hpre double-buffering throttles
        # it to one-slot-ahead execution), so each gelu's input is ready
        # with a full slot of margin and the ACT engine never gaps.
        for b in _SLOT_BLOCKS[0] + _SLOT_BLOCKS[1]:
            l1(b)
        for s in range(NS):
            gelu(s)
            if s >= 1:
                for b in _SLOT_BLOCKS[s - 1]:
                    l2(b)
            if s + 2 < NS:
                for b in _SLOT_BLOCKS[s + 2]:
                    l1(b)
        for b in _SLOT_BLOCKS[NS - 1]:
            l2(b)

    return nc


def make_weights_head(W1, b1, W2):
    """[128, CW] fp16 weight header for the head DMA."""
    head = np.zeros((128, CW), dtype=np.float16)
    w1u = np.asarray(W1[:32, :], dtype=np.float16)
    for band in range(4):
        head[32 * band:32 * band + 32, 0:128] = w1u
    head[:, 128:154] = np.asarray(W2[:, :26], dtype=np.float16)
    head[:, 160:162] = (
        np.asarray(b1, dtype=np.float32).reshape(128, 1).view(np.float16)
    )
    return head


def make_bit_planes(virtual_addr):
    """Per-core [NGRP_IN, 128, 2048] fp16 bit planes.

    Partition 32*(b%4)+i, col (l//4)*512 + c = bit i of addr
    (16k + l)*512 + c, where b = 16k + l.
    """
    va32 = np.asarray(virtual_addr).astype(np.uint32)
    out = []
    for c in range(NCORES):
        seg = va32[c * PER:(c + 1) * PER]
        byt = seg.view(np.uint8).reshape(NGRP_IN, 4, 4, BLK, 4)
        bits = np.unpackbits(byt, axis=-1, bitorder="little")  # [16,4j,4band,512,32]
        pl = bits.transpose(0, 2, 4, 1, 3).reshape(NGRP_IN, 128, 4 * BLK)
        out.append(np.ascontiguousarray(pl, dtype=np.float16))
    return out


def extract_logits(o):
    """[NOUT, 128, GOUT_G*BLK] f32 -> [PER, 26] logits in addr order.

    Rows past NB_DEV*BLK are garbage; the host fixup overwrites them.
    """
    arr = o.reshape(NOUT, 4, 32, GOUT_G, BLK)          # [o, band, i, j, c]
    l = arr.transpose(0, 3, 1, 4, 2).reshape(-1, 32)    # [(o,j,band,c), i]
    pad = PER - l.shape[0]
    if pad > 0:
        l = np.concatenate([l, np.zeros((pad, 32), l.dtype)])
    return l[:PER, :26]


_ERF = None


def _erf(x):
    global _ERF
    if _ERF is None:
        try:
            from scipy.special import erf as _e
            _ERF = _e
        except ImportError:
            _ERF = np.vectorize(math.erf)
    return _ERF(x)


def _fixup(logits, va, W1, b1, W2, b2):
    """Recompute near-threshold addrs (and the host-owned tail) in f64."""
    near = np.abs(logits - 0.5) < FIX_BAND
    rows = np.nonzero(near.any(axis=1))[0]
    rows = np.union1d(rows, np.arange(NB_DEV * BLK, PER))
    a = np.asarray(va)[rows].astype(np.int64)
    shifts = np.arange(32, dtype=np.int64)
    bits = ((a[:, None] >> shifts[None, :]) & 1).astype(np.float64)
    W1d = np.asarray(W1[:32, :], dtype=np.float64)
    hpre = bits @ W1d + np.asarray(b1, dtype=np.float64)
    h = 0.5 * hpre * (1.0 + _erf(hpre / np.sqrt(2.0)))
    lg = h @ np.asarray(W2[:, :26], dtype=np.float64) + np.asarray(
        b2[:26], dtype=np.float64
    )
    out = logits.copy()
    out[rows] = lg.astype(np.float32)
    return out, rows


_NC_CACHE = {}
TRACE = False
LAST_RES = None


def kernel(virtual_addr, W1, b1, W2, b2):
    global LAST_RES
    if "nc" not in _NC_CACHE:
        nc = build_nc()
        nc.finalize()
        _NC_CACHE["nc"] = nc
    nc = _NC_CACHE["nc"]

    whead = make_weights_head(W1, b1, W2)
    planes = make_bit_planes(virtual_addr)
    in_maps = []
    for c in range(NCORES):
        bp0 = np.concatenate([planes[c][0, :, 0:BLK], whead], axis=1)
        in_maps.append({"bp": planes[c], "bp0": np.ascontiguousarray(bp0)})

    res = bass_utils.run_bass_kernel_spmd(
        nc, in_maps, list(range(NCORES)), trace=TRACE
    )
    LAST_RES = res

    weights = np.int64(1) << np.arange(26, dtype=np.int64)
    b2f = np.asarray(b2[:26], dtype=np.float32)
    outs = []
    for c in range(NCORES):
        logits = extract_logits(res.results[c]["outp"]) + b2f
        va_core = np.asarray(virtual_addr)[c * PER:(c + 1) * PER]
        logits, _ = _fixup(logits, va_core, W1, b1, W2, b2)
        phys_bits = (logits > 0.5).astype(np.int64)
        outs.append(phys_bits @ weights)
    return np.concatenate(outs)


# revision 20
# speedup vs baseline: 1.0043x; 1.0043x over previous
"""NeuralMMU Trainium2 kernel (ship-logits design).

Per core (131072 addrs = 256 blocks of 512; device computes blocks 0-251,
the host computes the final 4 blocks exactly in f64 alongside the fixup):
  1. Host unpacks addresses into fp16 bit planes [128, cols]: partition
     32*(b%4)+k holds bit k of block b; one [128, 2048] DMA per 16 blocks.
     The first DMA also carries the fp16 weights (W1 replicated per band,
     W2, b1) so the whole head is a single short DMA chain.
  2. L1 per block: one fp16 matmul (K=32, 1 cyc/row): W1 @ bits ->
     PSUM hpre slot (f32 accumulate).  hpre slots cover 3 blocks
     ([128, 1536] = 3 PSUM banks, double buffered).
  3. ACT: exact Gelu(+b1) per slot, [128, 1536] PSUM -> SBUF fp16 h.
     This is the bottleneck engine (~124.5us busy of the ~132.6us total;
     the activation cost model is (N + 222 init)/1.2GHz per instruction,
     and PSUM capacity (8 banks) caps the slot size at 3 banks once hpre
     is double-buffered and l2o ping-pongs in the last 2 banks).
  4. L2 per block: one fp16 matmul h @ W2ext -> l2o [32 band, 512]
     (col tile_position), 4 blocks share a PSUM bank pair (2 banks,
     ping-pong).
  5. DVE copies l2o [128, 512] f32 PSUM -> fp16 SBUF; one [128, 1024]
     fp16 DMA per 2 groups ships raw logits to DRAM (the final group
     ships alone so the kernel tail is short).  fp16 quantization of the
     logits (+-1.2e-4) is far inside the host fixup band.
  6. Host adds b2, thresholds logits at 0.5 and packs the 26 bits;
     addrs with any logit within 4e-3 of 0.5 - plus the last 2048 addrs
     per core - are recomputed exactly in f64 (vectorized numpy).  The
     fp16 rounding error is ~2e-4 rms in logit space, so the band covers
     it with 20x margin and the output matches the f32 reference up to
     ~1e-7 ties.

PSUM budget: 2*3 (hpre) + 2*1 (l2o) = 8 banks.
Software pipeline per gelu-slot s: gelu(s) | L2+copy of slot s-1 |
L1 of slot s+2 (hpre double-buffering throttles L1 to one slot ahead in
execution), so each gelu's input is ready a full slot early and the ACT
engine runs back-to-back.  Dummy matmuls at t=0 warm the PE p-state; a
dummy activation pulls the gelu table load off the critical path.
"""

import math

import numpy as np
from contextlib import ExitStack

import concourse.bass as bass
import concourse.mybir as mybir
import concourse.tile as tile
from concourse import bacc, bass_utils

B = 1_048_576
NCORES = 8
PER = B // NCORES          # 131072 addrs per core
BLK = 512                  # addrs per block (one matmul)
NB = PER // BLK            # 256 blocks per core
NB_DEV = 252               # blocks computed on device (last 4 on host)
GIN_B = 16                 # blocks per input DMA ([128, 2048] fp16)
NGRP_IN = NB // GIN_B      # 16 input DMAs
NGROUP = NB_DEV // 4       # 63 logit groups (one l2o bank each)
GOUT_G = 2                 # groups per output DMA ([128, 1024] f32)
NOUT = (NGROUP + GOUT_G - 1) // GOUT_G   # 32 output DMA slots
CW = 162                   # fp16 cols of weights in the head DMA
# gelu slot sizes in blocks (sum 252): tiny head slot so the ACT engine
# starts early, small tail slot so the kernel drains fast.
SLOT_SIZES = [1] + [3] * 83 + [2]
NS = len(SLOT_SIZES)       # 85

_SLOT_OF = []
for _s, _sz in enumerate(SLOT_SIZES):
    for _p in range(_sz):
        _SLOT_OF.append((_s, _p))
_SLOT_BLOCKS = [[] for _ in range(NS)]
for _b, (_s, _p) in enumerate(_SLOT_OF):
    _SLOT_BLOCKS[_s].append(_b)

F32 = mybir.dt.float32
F16 = mybir.dt.float16
AF = mybir.ActivationFunctionType

FIX_BAND = 4e-3            # host recomputes addrs with |logit-0.5| < FIX_BAND


def build_nc() -> bass.Bass:
    nc = bacc.Bacc("TRN2")

    bp = nc.dram_tensor("bp", [NGRP_IN, 128, (GIN_B // 4) * BLK], F16,
                        kind="ExternalInput")
    # Head DMA payload: blocks 0-3 (cols 0:512) + fp16 weights (cols 512:512+CW)
    bp0 = nc.dram_tensor("bp0", [128, BLK + CW], F16, kind="ExternalInput")
    outp = nc.dram_tensor("outp", [NOUT, 128, GOUT_G * BLK], F16,
                          kind="ExternalOutput")

    with ExitStack() as ctx:
        tc = ctx.enter_context(tile.TileContext(nc))
        const = ctx.enter_context(tc.tile_pool(name="const", bufs=1))
        rp = ctx.enter_context(tc.tile_pool(name="rp", bufs=2))
        hp = ctx.enter_context(tc.tile_pool(name="hp", bufs=3))
        lop = ctx.enter_context(tc.tile_pool(name="lop", bufs=2))
        hprep = ctx.enter_context(tc.tile_pool(name="hprep", bufs=2, space="PSUM"))
        l2p = ctx.enter_context(tc.tile_pool(name="l2p", bufs=2, space="PSUM"))

        R = {}                     # input-group index -> tile
        hpre_t = {}                # slot -> PSUM tile
        h_t = {}                   # slot -> SBUF tile
        l2o_t = {}                 # group -> PSUM tile
        lout_t = {}                # out-DMA index -> SBUF tile

        R0x = const.tile([128, BLK + CW], F16, name="R0x")
        nc.sync.dma_start(R0x[:], bp0[:])
        w1r = R0x[:, BLK:BLK + 128]       # W1 fp16, replicated x4 bands
        w2s = R0x[:, BLK + 128:BLK + 160]  # W2[:, :26] fp16, padded to 32
        b1c = R0x[:, BLK + 160:BLK + 162].bitcast(F32)  # b1 f32 [128, 1]

        R0b = rp.tile([128, 3 * BLK], F16, name="R0b", tag="r0b")
        nc.sync.dma_start(R0b[:], bp[0, :, BLK:4 * BLK])

        # PE p-state warm-up: dummy matmuls on a zeroed tile keep the PE
        # busy from ~t=0 until the first real L1 (which lands right after
        # the single head DMA), so real matmuls start past the cold clock.
        wz = const.tile([32, BLK], F32, name="wz")
        nc.vector.memset(wz[:], 0.0)
        # Dummy first activation with no DMA deps: bacc inserts the gelu
        # table load before it, so the 1.3us load runs at ~1us instead of
        # waiting behind gelu(0)'s head-DMA semaphore.
        wgelu = const.tile([32, 1], F32, name="wgelu")
        nc.scalar.activation(wgelu[:], wz[:, 0:1], AF.Gelu, bias=0.0, scale=1.0)
        hpre_t[0] = hprep.tile([128, 3 * BLK], F32, name="hpre")
        for i in range(4):
            nc.tensor.matmul(
                hpre_t[0][:, BLK:2 * BLK][0:128, 0:128],
                wz[:, 0:128], wz[:, 0:128],
                start=True, stop=True, tile_position=(0, 0),
            )

        def dma_in(k):
            t = rp.tile([128, (GIN_B // 4) * BLK], F16)
            nc.sync.dma_start(t[:], bp[k])
            R[k] = t

        def l1(b):
            k, l = divmod(b, GIN_B)
            if l == 0 and k + 1 < NGRP_IN:
                dma_in(k + 1)
            band = b % 4
            s, pos = _SLOT_OF[b]
            if pos == 0 and s not in hpre_t:
                hpre_t[s] = hprep.tile([128, 3 * BLK], F32, name="hpre")
            col = (l // 4) * BLK
            if k == 0 and l < 4:
                src = R0x[32 * band:32 * band + 32, 0:BLK]
            elif k == 0:
                src = R0b[32 * band:32 * band + 32, col - BLK:col]
            else:
                src = R[k][32 * band:32 * band + 32, col:col + BLK]
            nc.tensor.matmul(
                hpre_t[s][:, pos * BLK:(pos + 1) * BLK],
                w1r[32 * band:32 * band + 32, :],
                src,
                start=True, stop=True, tile_position=(32 * band, 0),
            )

        def gelu(s):
            n = SLOT_SIZES[s] * BLK
            ht = hp.tile([128, 3 * BLK], F16)
            nc.scalar.activation(ht[:, 0:n], hpre_t[s][:, 0:n], AF.Gelu,
                                 bias=b1c, scale=1.0)
            h_t[s] = ht

        def l2(b):
            band = b % 4
            g = b // 4
            s, pos = _SLOT_OF[b]
            if band == 0:
                l2o_t[g] = l2p.tile([128, BLK], F32, name="l2o")
            nc.tensor.matmul(
                l2o_t[g][32 * band:32 * band + 32, :],
                w2s,
                h_t[s][:, pos * BLK:(pos + 1) * BLK],
                start=True, stop=True, tile_position=(0, 32 * band),
            )
            if band == 3:
                o, j = divmod(g, GOUT_G)
                if j == 0:
                    lout_t[o] = lop.tile([128, GOUT_G * BLK], F16, name="lout")
                nc.vector.tensor_copy(
                    lout_t[o][:, j * BLK:(j + 1) * BLK], l2o_t[g][:]
                )
                if g == NGROUP - 1:
                    # The final group ships alone so the kernel tail only
                    # waits on a short [128, 512] DMA.
                    nc.sync.dma_start(
                        outp[o][:, j * BLK:(j + 1) * BLK],
                        lout_t[o][:, j * BLK:(j + 1) * BLK],
                    )
                elif j == GOUT_G - 1:
                    nc.sync.dma_start(outp[o], lout_t[o][:])

        # L1 runs two slots ahead of gelu (hpre double-buffering throttles
        # it to one-slot-ahead execution), so each gelu's input is ready
        # with a full slot of margin and the ACT engine never gaps.
        for b in _SLOT_BLOCKS[0] + _SLOT_BLOCKS[1]:
            l1(b)
        for s in range(NS):
            gelu(s)
            if s >= 1:
                for b in _SLOT_BLOCKS[s - 1]:
                    l2(b)
            if s + 2 < NS:
                for b in _SLOT_BLOCKS[s + 2]:
                    l1(b)
        for b in _SLOT_BLOCKS[NS - 1]:
            l2(b)

    return nc


def make_weights_head(W1, b1, W2):
    """[128, CW] fp16 weight header for the head DMA."""
    head = np.zeros((128, CW), dtype=np.float16)
    w1u = np.asarray(W1[:32, :], dtype=np.float16)
    for band in range(4):
        head[32 * band:32 * band + 32, 0:128] = w1u
    head[:, 128:154] = np.asarray(W2[:, :26], dtype=np.float16)
    head[:, 160:162] = (
        np.asarray(b1, dtype=np.float32).reshape(128, 1).view(np.float16)
    )
    return head


def make_bit_planes(virtual_addr):
    """Per-core [NGRP_IN, 128, 2048] fp16 bit planes.

    Partition 32*(b%4)+i, col (l//4)*512 + c = bit i of addr
    (16k + l)*512 + c, where b = 16k + l.
    """
    va32 = np.asarray(virtual_addr).astype(np.uint32)
    out = []
    for c in range(NCORES):
        seg = va32[c * PER:(c + 1) * PER]
        byt = seg.view(np.uint8).reshape(NGRP_IN, 4, 4, BLK, 4)
        bits = np.unpackbits(byt, axis=-1, bitorder="little")  # [16,4j,4band,512,32]
        pl = bits.transpose(0, 2, 4, 1, 3).reshape(NGRP_IN, 128, 4 * BLK)
        out.append(np.ascontiguousarray(pl, dtype=np.float16))
    return out


def extract_logits(o):
    """[NOUT, 128, GOUT_G*BLK] fp16 -> [PER, 26] f32 logits in addr order.

    Rows past NB_DEV*BLK are garbage; the host fixup overwrites them.
    """
    arr = o.astype(np.float32).reshape(NOUT, 4, 32, GOUT_G, BLK)  # [o, band, i, j, c]
    l = arr.transpose(0, 3, 1, 4, 2).reshape(-1, 32)    # [(o,j,band,c), i]
    pad = PER - l.shape[0]
    if pad > 0:
        l = np.concatenate([l, np.zeros((pad, 32), l.dtype)])
    return l[:PER, :26]


_ERF = None


def _erf(x):
    global _ERF
    if _ERF is None:
        try:
            from scipy.special import erf as _e
            _ERF = _e
        except ImportError:
            _ERF = np.vectorize(math.erf)
    return _ERF(x)


def _fixup(logits, va, W1, b1, W2, b2):
    """Recompute near-threshold addrs (and the host-owned tail) in f64."""
    near = np.abs(logits - 0.5) < FIX_BAND
    rows = np.nonzero(near.any(axis=1))[0]
    rows = np.union1d(rows, np.arange(NB_DEV * BLK, PER))
    a = np.asarray(va)[rows].astype(np.int64)
    shifts = np.arange(32, dtype=np.int64)
    bits = ((a[:, None] >> shifts[None, :]) & 1).astype(np.float64)
    W1d = np.asarray(W1[:32, :], dtype=np.float64)
    hpre = bits @ W1d + np.asarray(b1, dtype=np.float64)
    h = 0.5 * hpre * (1.0 + _erf(hpre / np.sqrt(2.0)))
    lg = h @ np.asarray(W2[:, :26], dtype=np.float64) + np.asarray(
        b2[:26], dtype=np.float64
    )
    out = logits.copy()
    out[rows] = lg.astype(np.float32)
    return out, rows


_NC_CACHE = {}
TRACE = False
LAST_RES = None


def kernel(virtual_addr, W1, b1, W2, b2):
    global LAST_RES
    if "nc" not in _NC_CACHE:
        nc = build_nc()
        nc.finalize()
        _NC_CACHE["nc"] = nc
    nc = _NC_CACHE["nc"]

    whead = make_weights_head(W1, b1, W2)
    planes = make_bit_planes(virtual_addr)
    in_maps = []
    for c in range(NCORES):
        bp0 = np.concatenate([planes[c][0, :, 0:BLK], whead], axis=1)
        in_maps.append({"bp": planes[c], "bp0": np.ascontiguousarray(bp0)})

    res = bass_utils.run_bass_kernel_spmd(
        nc, in_maps, list(range(NCORES)), trace=TRACE
    )
    LAST_RES = res

    weights = np.int64(1) << np.arange(26, dtype=np.int64)
    b2f = np.asarray(b2[:26], dtype=np.float32)
    outs = []
    for c in range(NCORES):
        logits = extract_logits(res.results[c]["outp"]) + b2f
        va_core = np.asarray(virtual_addr)[c * PER:(c + 1) * PER]
        logits, _ = _fixup(logits, va_core, W1, b1, W2, b2)
        phys_bits = (logits > 0.5).astype(np.int64)
        outs.append(phys_bits @ weights)
    return np.concatenate(outs)
